# revision 29
# baseline (speedup 1.0000x reference)
# PRoPE attention Trainium2 kernel.
# Sharding: 8 cores = 2 batches x 4 head-groups (4 heads each).
# v3 design:
#   Phase 1 (projections + PRoPE) in bf16, engine-balanced: ScalarE moves
#   PSUM->SBUF, DVE does shuffle+add (+every other mult), GpSimd the other
#   mult, PE stays dense (proj + per-cam block matmuls + v transposes).
#   Phase 2 (attention) is software-pipelined per k-tile, ACT-bound:
#   PE issues av(t) and scores(t+1) right after exp(t). PSUM: 2 score
#   slots (128,1024) + 2 softmax accumulators = exactly 8 banks.
#   Group boundaries: the next group's S/E prologue is emitted BEFORE the
#   previous group's tail (Mo + normalize), so the PE never idles long
#   enough to re-trigger the HAM clock throttle. Mo camera matmuls write
#   into po[1]'s banks. Denominator: ones-column in v -> DVE reciprocal ->
#   2-hop DRAM bounce broadcast. Output projection at the end reuses the
#   score-slot rotation, partials stored bf16, summed on host.
import functools

import numpy as np

B, L, DM = 2, 2048, 1024
H, DH, NG = 16, 64, 16          # heads, head_dim, groups of 4
CAMS, PER_CAM = 8, 256
PX, PY, IW, IH, ROPE_BASE = 16, 16, 256.0, 256.0, 10000.0
HPG = 4                          # heads per group (per core)
HD4 = HPG * DH                   # 256 cols of qkv per core
N_CORES = 8


def _rope_tables():
    """C and S~ tables in (Dh, PER_CAM) layout, tiled to (128, L)."""
    idx = np.arange(PER_CAM)
    u = ((idx % PX) + 0.5) * (IW / PX)
    v = ((idx // PX) + 0.5) * (IH / PY)
    freqs = (np.float32(ROPE_BASE) ** (-(np.arange(NG, dtype=np.float32)) / np.float32(NG)))
    tu = (u[:, None] * freqs[None, :]).astype(np.float32)   # (P, G)
    tv = (v[:, None] * freqs[None, :]).astype(np.float32)
    ca, sa, cb, sb = np.cos(tu), np.sin(tu), np.cos(tv), np.sin(tv)
    Cq = np.zeros((DH, PER_CAM), np.float32)
    Sq = np.zeros((DH, PER_CAM), np.float32)
    for g in range(NG):
        Cq[4 * g + 0] = ca[:, g]; Cq[4 * g + 1] = ca[:, g]
        Cq[4 * g + 2] = cb[:, g]; Cq[4 * g + 3] = cb[:, g]
        Sq[4 * g + 0] = -sa[:, g]; Sq[4 * g + 1] = sa[:, g]
        Sq[4 * g + 2] = -sb[:, g]; Sq[4 * g + 3] = sb[:, g]
    CqL = np.tile(Cq, (1, CAMS))           # (64, 2048)
    SqL = np.tile(Sq, (1, CAMS))
    return np.tile(CqL, (2, 1)), np.tile(SqL, (2, 1))  # (128, 2048)


def _cam_mats(viewmats, Ks):
    """Per-(batch,cam) P and P^-1."""
    K4 = np.zeros((B, CAMS, 4, 4), np.float32)
    K4[..., :3, :3] = Ks
    K4[..., 3, 3] = 1.0
    P = (K4 @ viewmats).astype(np.float32)
    P_inv = np.linalg.inv(P.astype(np.float64)).astype(np.float32)
    return P, P_inv


@functools.lru_cache(maxsize=1)
def _build_nc():
    import concourse.bass as bass
    import concourse.mybir as mybir
    from concourse.tile import TileContext
    from contextlib import ExitStack

    dt = mybir.dt
    f32 = dt.float32
    f32r = dt.float32r
    bf16 = dt.bfloat16
    ALU = mybir.AluOpType
    ACT = mybir.ActivationFunctionType

    nc = bass.Bass("TRN2", target_bir_lowering=False, debug=False,
                   num_devices=N_CORES)

    xT_d = nc.dram_tensor("xt", [DM, L], bf16, kind="ExternalInput")
    wq_d = nc.dram_tensor("wq", [DM, HD4], bf16, kind="ExternalInput")
    wk_d = nc.dram_tensor("wk", [DM, HD4], bf16, kind="ExternalInput")
    wv_d = nc.dram_tensor("wv", [DM, HD4], bf16, kind="ExternalInput")
    wp_d = nc.dram_tensor("wp", [HD4, DM], bf16, kind="ExternalInput")
    cscf_d = nc.dram_tensor("cscf", [128, L], f32, kind="ExternalInput")
    cssf_d = nc.dram_tensor("cssf", [128, L], f32, kind="ExternalInput")
    camq_d = nc.dram_tensor("camq", [CAMS, 128, 128], f32, kind="ExternalInput")
    camk_d = nc.dram_tensor("camk", [CAMS, 128, 128], f32, kind="ExternalInput")
    camo_d = nc.dram_tensor("camo", [CAMS, 128, 128], f32, kind="ExternalInput")
    ident_d = nc.dram_tensor("ident", [128, 128], bf16, kind="ExternalInput")
    yp_d = nc.dram_tensor("yp", [L, DM], bf16, kind="ExternalOutput")
    sel_d = nc.dram_tensor("sel", [128, 128], f32, kind="ExternalInput")
    bounce_d = nc.dram_tensor("bounce", [8, 1024], f32, kind="Internal")
    bounce2_d = nc.dram_tensor("bounce2", [8, 1024], f32, kind="Internal")

    SWAP_MASK = [i ^ 1 for i in range(32)]

    with TileContext(nc) as tc, ExitStack() as ctx:
        # ---- persistent pools --------------------------------------------
        ptab = ctx.enter_context(tc.tile_pool(name="ptab", bufs=6))
        pcam = ctx.enter_context(tc.tile_pool(name="pcam", bufs=40))
        pqk = ctx.enter_context(tc.tile_pool(name="pqk", bufs=4))
        pvt = ctx.enter_context(tc.tile_pool(name="pvt", bufs=32))
        popt = ctx.enter_context(tc.tile_pool(name="popt", bufs=2))
        pwp = ctx.enter_context(tc.tile_pool(name="pwp", bufs=2))

        # DMA priority: wk -> xt0 -> camk -> tables -> wv -> ...
        cscf = ptab.tile([128, L], f32, tag="cscf", bufs=1)
        cssf = ptab.tile([128, L], f32, tag="cssf", bufs=1)
        ident = ptab.tile([128, 128], bf16, tag="id", bufs=1)

        sel = ptab.tile([128, 128], f32r, tag="sel", bufs=1)
        nc.sync.dma_start(sel[:], sel_d[:, :].bitcast(f32r))
        rcpl = ptab.tile([128, 1024], f32r, tag="rcpl", bufs=1)
        nc.vector.tensor_copy(rcpl[:], cscf[:, 0:1024])
        qp = [pqk.tile([128, L], bf16, tag="qk", bufs=4, name=f"qp{i}") for i in range(2)]
        kp = [pqk.tile([128, L], bf16, tag="qk", bufs=4, name=f"kp{i}") for i in range(2)]
        # v tiles (tok, dh): [A64 | 1 | B64 | 1] = 130 cols
        vt = [[pvt.tile([128, 130], bf16, tag="vt", bufs=32, name=f"vt{i}_{t}") for t in range(16)]
              for i in range(2)]
        opT = [popt.tile([128, L], bf16, tag="opt", bufs=2, name=f"opT{i}") for i in range(2)]

        with tc.tile_pool(name="pxt", bufs=32) as pxt, \
             tc.tile_pool(name="pw", bufs=24) as pw, \
             tc.tile_pool(name="pvp", bufs=2) as pvp, \
             tc.tile_pool(name="ptmp", bufs=16) as ptmp, \
             tc.tile_pool(name="psP", bufs=7, space="PSUM") as psP:

            # ---- phase 1: projections + PRoPE ----------------------------
            def big_w(name, dram):
                tile = pw.tile([128, 8 * HD4], bf16, tag="w", bufs=3, name=name)
                nc.sync.dma_start(
                    tile[:].rearrange("p (d f) -> p d f", d=8),
                    dram[:, :].rearrange("(d p) f -> p d f", d=8))
                return tile

            def big_cam(name, dram):
                tile = pcam.tile([128, 8 * 128], f32r, tag="cam", bufs=4, name=name)
                nc.sync.dma_start(
                    tile[:].rearrange("p (c f) -> p c f", c=8),
                    dram[:, :, :].rearrange("c p f -> p c f").bitcast(f32r))
                return tile

            wk_a = big_w("wk", wk_d)
            xt = [pxt.tile([128, 4096], bf16, tag="xt", bufs=4, name=f"xt{lb}")
                  for lb in range(4)]
            nc.sync.dma_start(
                xt[0][:].rearrange("p (d f) -> p d f", d=8),
                xT_d[:, 0:512].rearrange("(d p) f -> p d f", d=8))
            camk = big_cam("camk", camk_d)
            nc.sync.dma_start(ident[:], ident_d[:, :])
            nc.sync.dma_start(cscf[:], cscf_d[:, :])
            nc.sync.dma_start(cssf[:], cssf_d[:, :])
            wv_a = big_w("wv", wv_d)
            nc.sync.dma_start(
                xt[1][:].rearrange("p (d f) -> p d f", d=8),
                xT_d[:, 512:1024].rearrange("(d p) f -> p d f", d=8))
            camq = big_cam("camq", camq_d)
            wq_a = big_w("wq", wq_d)
            for lb in range(2, 4):
                nc.sync.dma_start(
                    xt[lb][:].rearrange("p (d f) -> p d f", d=8),
                    xT_d[:, 512 * lb:512 * lb + 512].rearrange(
                        "(d p) f -> p d f", d=8))
            camo = big_cam("camo", camo_d)
            wp = [pwp.tile([128, DM], bf16, tag="wp", bufs=2, name=f"wp{i}") for i in range(2)]
            for pt in range(2):
                nc.sync.dma_start(wp[pt][:], wp_d[128 * pt:128 * pt + 128, :])
            # preload the exp table set with a dummy 1-elem activation
            dumm = ptmp.tile([1, 2], bf16, tag="dumm", bufs=1)
            nc.scalar.activation(dumm[:, 0:1], cscf[0:1, 0:1], ACT.Exp, scale=0.125)
            nc.scalar.activation(dumm[:, 1:2], cscf[0:1, 0:1], ACT.Ln, scale=1.0)

            vp = [pvp.tile([128, L], bf16, tag="vp", bufs=2, name=f"vp{i}") for i in range(2)]

            def prope_block(w8, dest, camm, pt, lb):
                """One (tensor, pt, 512-token) projection + PRoPE block.
                All rotation ops on DVE in f32 (bf16 TT-mult and GpSimd both
                hit slow paths); single camera matmul pair per 256-token cam."""
                lsl = slice(512 * lb, 512 * lb + 512)
                acc = psP.tile([128, 512], f32, tag="acc", bufs=4)
                for d in range(8):
                    wsl = slice(256 * d + 128 * pt, 256 * d + 128 * pt + 128)
                    nc.tensor.matmul(acc[:], w8[:, wsl],
                                     xt[lb][:, 512 * d:512 * d + 512],
                                     start=(d == 0), stop=(d == 7))
                tb = ptmp.tile([128, 512], f32, tag="tb", bufs=3)
                nc.scalar.copy(tb[:], acc[:])
                sw = ptmp.tile([128, 512], f32, tag="sw", bufs=3)
                nc.vector.stream_shuffle(sw[:], tb[:], SWAP_MASK)
                t1 = ptmp.tile([128, 512], f32, tag="t1", bufs=3)
                nc.vector.tensor_tensor(t1[:], cscf[:, lsl], tb[:], op=ALU.mult)
                sw2 = ptmp.tile([128, 512], f32, tag="sw2", bufs=3)
                nc.vector.tensor_tensor(sw2[:], cssf[:, lsl], sw[:], op=ALU.mult)
                dd = ptmp.tile([128, 512], f32r, tag="dd", bufs=3)
                nc.vector.tensor_tensor(dd[:], t1[:], sw2[:], op=ALU.add)
                cc = psP.tile([128, 512], f32, tag="cc", bufs=2)
                for ci in range(2):
                    cam = 2 * lb + ci
                    csl = slice(256 * ci, 256 * ci + 256)
                    nc.tensor.matmul(cc[:, csl],
                                     camm[:, 128 * cam:128 * cam + 128],
                                     dd[:, csl], start=True, stop=True)
                nc.scalar.copy(dest[pt][:, lsl], cc[:])

            # k first (both pt), then v (+ transposes), then q
            for lb in range(4):
                for pt in range(2):
                    prope_block(wk_a, kp, camk, pt, lb)
            for lb in range(4):
                for pt in range(2):
                    prope_block(wv_a, vp, camk, pt, lb)
                    for t in range(4 * lb, 4 * lb + 4):
                        dst = vt[pt][t]
                        nc.gpsimd.memset(dst[:], 1.0)
                        tp = psP.tile([128, 128], bf16, tag="tp", bufs=2)
                        nc.tensor.transpose(tp[:], vp[pt][:, 128 * t:128 * t + 128],
                                            ident[:])
                        nc.vector.tensor_copy(dst[:, 0:64], tp[:, 0:64])
                        nc.vector.tensor_copy(dst[:, 65:129], tp[:, 64:128])
            for bi, lb in enumerate((2, 3, 0, 1)):
                for pt in range(2):
                    if bi in (1, 3) and pt == 0:
                        wacc = psP.tile([128, 512], f32, tag="acc", bufs=4)
                        for _ in range(12):
                            nc.tensor.matmul(wacc[:], ident[:],
                                             xt[lb][:, 0:512],
                                             start=True, stop=True)
                    prope_block(wq_a, qp, camq, pt, lb)
            wacc = psP.tile([128, 512], f32, tag="acc", bufs=4)
            for _ in range(14):
                nc.tensor.matmul(wacc[:], ident[:], xt[0][:, 0:512],
                                 start=True, stop=True)

        with tc.tile_pool(name="pat", bufs=8) as pat, \
             tc.tile_pool(name="psm", bufs=16) as psm, \
             tc.tile_pool(name="pyo", bufs=4) as pyo, \
             tc.tile_pool(name="psS", bufs=2, space="PSUM") as psS, \
             tc.tile_pool(name="psO", bufs=2, space="PSUM") as psO:

            # ---- phase 2: attention, ACT-bound software pipeline ---------
            def S(pt, qg, t, hi, pstiles, warm=False):
                """Scores for k-tile t, head hi: ps = kp_slice.T @ qp (2 MMs).
                warm=True first issues a full-array 128x128 dummy matmul into
                the slot (overwritten by the real scores): the HAM clock gate
                reads the half-array attention matmuls as ~50% activity and
                decays to 1.2 GHz; a full-array MAC burst re-arms 2.4 GHz for
                ~40us."""
                hsl = slice(64 * hi, 64 * hi + 64)
                ksl = slice(128 * t, 128 * t + 128)
                ps = psS.tile([128, 1024], f32, tag="sc", bufs=2,
                              name=f"sc{pt}_{qg}_{t}_{hi}")
                if warm:
                    for _ in range(6):
                        nc.tensor.matmul(ps[:, 0:512], ident[:],
                                         qp[pt][:, 0:512],
                                         start=True, stop=True)
                for qh in range(2):
                    qsl = slice(1024 * qg + 512 * qh, 1024 * qg + 512 * qh + 512)
                    nc.tensor.matmul(ps[:, 512 * qh:512 * qh + 512],
                                     kp[pt][hsl, ksl], qp[pt][hsl, qsl],
                                     start=True, stop=True,
                                     tile_position=(64 * hi, 0))
                pstiles[(t, hi)] = ps

            def E(pt, qg, t, hi, pstiles, attiles):
                at = pat.tile([128, 1024], bf16, tag="at", bufs=8,
                              name=f"at{pt}_{qg}_{t}_{hi}")
                nc.scalar.activation(at[:], pstiles[(t, hi)][:], ACT.Exp,
                                     scale=0.125)
                attiles[(t, hi)] = at

            def A(pt, qg, t, hi, po, attiles):
                for qh in range(2):
                    nc.tensor.matmul(
                        po[hi][0:65, 512 * qh:512 * qh + 512],
                        vt[pt][t][:, 65 * hi:65 * hi + 65],
                        attiles[(t, hi)][:, 512 * qh:512 * qh + 512],
                        start=(t == 0), stop=(t == 15))

            def tail_a(g, pt, qg, po, st, last=False):
                """Denominators + oc copies for a finished group (DVE/DMA).
                Reciprocal runs at 8 cyc/elem per lane, so reshape the (1,1024)
                denominator row to (128,8) via a DRAM bounce before it."""
                oc = psm.tile([128, 1024], f32r, tag="oc", bufs=2, name=f"oc{g}")
                if last:
                    # ScalarE is idle after the final exps: 1/den = exp(-ln d),
                    # broadcast on PE via the selector - no DMA latency chain.
                    dn = psm.tile([128, 1024], f32, tag="dnl", bufs=1, name="dnl")
                    lg = psm.tile([128, 1024], f32, tag="lgl", bufs=1, name="lgl")
                    for hi in range(2):
                        r = 64 * hi
                        nc.scalar.copy(oc[64 * hi:64 * hi + 64, :],
                                       po[hi][0:64, :])
                        nc.vector.tensor_copy(dn[r:r + 1, :],
                                              po[hi][64:65, :])
                        nc.scalar.activation(lg[r:r + 1, :], dn[r:r + 1, :],
                                             ACT.Ln)
                        nc.scalar.activation(dn[r + 32:r + 33, :],
                                             lg[r:r + 1, :], ACT.Exp,
                                             scale=-1.0)
                        nc.vector.tensor_copy(rcpl[r:r + 1, :],
                                              dn[r + 32:r + 33, :])
                    rd = psO.tile([128, 1024], f32, tag="po", bufs=2,
                                  name="rdl")
                    for qh in range(2):
                        nc.tensor.matmul(
                            rd[:, 512 * qh:512 * qh + 512], sel[:],
                            rcpl[:, 512 * qh:512 * qh + 512],
                            start=True, stop=True)
                    st["oc"], st["rd"] = oc, rd
                    return
                rd = psm.tile([128, 1024], f32, tag="rd", bufs=2, name=f"rd{g}")
                for hi in range(2):
                    dn = psm.tile([1, 1024], f32, tag=f"dn{hi}", bufs=2,
                                  name=f"dn{g}_{hi}")
                    nc.vector.tensor_copy(oc[64 * hi:64 * hi + 64, :],
                                          po[hi][0:64, :])
                    nc.vector.tensor_copy(dn[:], po[hi][64:65, :])
                    nc.sync.dma_start(bounce_d[2 * g + hi, :][None, :], dn[:])
                    rc = psm.tile([128, 8], f32, tag=f"rc{hi}", bufs=2,
                                  name=f"rc{g}_{hi}")
                    nc.sync.dma_start(
                        rc[:], bounce_d[2 * g + hi:2 * g + hi + 1, :].rearrange(
                            "a (p f) -> (a p) f", p=128))
                    rc2 = psm.tile([128, 8], f32, tag=f"rc2{hi}", bufs=2,
                                   name=f"rc2{g}_{hi}")
                    nc.vector.reciprocal(rc2[:], rc[:])
                    nc.sync.dma_start(
                        bounce2_d[2 * g + hi, :][None, :].rearrange(
                            "a (p f) -> (a p) f", p=128), rc2[:])
                    nc.sync.dma_start(
                        rd[64 * hi:64 * hi + 64, :],
                        bounce2_d[2 * g + hi, :][None, :].to_broadcast((64, 1024)))
                st["oc"], st["rd"] = oc, rd

            def tail_b(g, pt, qg, po, st, split=False):
                """Mo camera matmuls + D^T rotation + normalize -> opT."""
                oc, rd = st["oc"], st["rd"]
                mo = psS.tile([128, 1024], f32, tag="sc", bufs=2,
                              name=f"mo{g}")
                halves = (range(2) if split else range(1))
                w = 1024 // len(list(halves))
                for qh in halves:
                    hl = slice(w * qh, w * qh + w)
                    gsl = slice(1024 * qg + w * qh, 1024 * qg + w * qh + w)
                    for ci in range(w // 256):
                        cam = 4 * qg + (w * qh) // 256 + ci
                        csl = slice(w * qh + 256 * ci, w * qh + 256 * ci + 256)
                        nc.tensor.matmul(mo[:, csl],
                                         camo[:, 128 * cam:128 * cam + 128],
                                         oc[:, csl].bitcast(f32r),
                                         start=True, stop=True)
                    sw = psm.tile([128, w], f32, tag=f"msw{qh}", bufs=2,
                                  name=f"msw{g}_{qh}")
                    nc.vector.stream_shuffle(sw[:], mo[:, hl], SWAP_MASK)
                    t1 = psm.tile([128, w], f32, tag=f"mt1{qh}", bufs=2,
                                  name=f"mt1{g}_{qh}")
                    nc.vector.tensor_tensor(t1[:], cscf[:, gsl], mo[:, hl],
                                            op=ALU.mult)
                    sw2 = psm.tile([128, w], f32, tag=f"msw2{qh}", bufs=2,
                                   name=f"msw2{g}_{qh}")
                    nc.vector.tensor_tensor(sw2[:], cssf[:, gsl], sw[:],
                                            op=ALU.mult)
                    t2 = psm.tile([128, w], f32, tag=f"mt2{qh}", bufs=2,
                                  name=f"mt2{g}_{qh}")
                    nc.vector.tensor_tensor(t2[:], t1[:], sw2[:],
                                            op=ALU.subtract)
                    nc.vector.tensor_tensor(opT[pt][:, gsl], t2[:], rd[:, hl],
                                            op=ALU.mult)

            def proj_lt(lt):
                tsl = slice(128 * lt, 128 * lt + 128)
                pool_, tag_ = ((psS, "sc") if lt % 2 == 0 else (psO, "po"))
                ys = pool_.tile([128, 1024], f32, tag=tag_, bufs=2, name=f"ys{lt}")
                for nb in range(2):
                    nsl = slice(512 * nb, 512 * nb + 512)
                    nc.tensor.matmul(ys[:, nsl], opT[0][:, tsl], wp[0][:, nsl],
                                     start=True, stop=False)
                    nc.tensor.matmul(ys[:, nsl], opT[1][:, tsl], wp[1][:, nsl],
                                     start=False, stop=True)
                yo = pyo.tile([128, 1024], bf16, tag="yo", bufs=4, name=f"yo{lt}")
                if lt % 2 == 0:
                    nc.scalar.copy(yo[:], ys[:])
                else:
                    nc.vector.tensor_copy(yo[:], ys[:])
                nc.sync.dma_start(yp_d[tsl, :], yo[:])

            groups = [(0, 1), (0, 0), (1, 1), (1, 0)]
            prev = None
            for g, (pt, qg) in enumerate(groups):
                po = [psO.tile([128, 1024], f32, tag="po", bufs=2,
                               name=f"po{g}_{hi}") for hi in range(2)]
                pstiles, attiles = {}, {}
                for hi in range(2):
                    S(pt, qg, 0, hi, pstiles)
                    E(pt, qg, 0, hi, pstiles, attiles)
                if prev is not None:
                    tail_a(*prev)
                for t in range(16):
                    if t == 0:
                        # dense dummy burst: fills the PE queue so it issues
                        # gap-free for a full HAM window and re-arms 2.4 GHz;
                        # av(0) start=True clears the bank afterwards
                        for _ in range(12):
                            nc.tensor.matmul(po[0][:, 0:512], ident[:],
                                             qp[pt][:, 0:512],
                                             start=True, stop=True)
                    for hi in range(2):
                        A(pt, qg, t, hi, po, attiles)
                        if t < 15:
                            S(pt, qg, t + 1, hi, pstiles,
                              warm=(hi == 0 and t + 1 in (5, 11)))
                            E(pt, qg, t + 1, hi, pstiles, attiles)
                    if t == 2 and prev is not None:
                        tail_b(*prev)
                prev = (g, pt, qg, po, {})
            # last group: tail + projection, overlapped
            tail_a(*prev, last=True)
            proj_lt(8)
            proj_lt(9)
            tail_b(*prev, split=True)
            for lt in range(10, 16):
                proj_lt(lt)
            for lt in range(0, 8):
                proj_lt(lt)

    return nc


def _split_multi_waits(nc):
    """This walrus build accepts only one sync-wait per instruction; move
    extras onto standalone InstEventSemaphore ops just before."""
    import concourse.mybir as mybir
    n = 0
    for f in nc.m.functions:
        for bb in f.blocks:
            new_insts = []
            for inst in bb.instructions:
                si = inst.sync_info
                if si is not None and si.on_wait and len(si.on_wait) > 1:
                    waits = list(si.on_wait)
                    for w in waits[:-1]:
                        n += 1
                        new_insts.append(mybir.InstEventSemaphore(
                            name=f"I-splitw-{n}", engine=inst.engine,
                            ins=[], outs=[],
                            sync_info=mybir.SyncInfo(on_wait=[w], on_update=[]),
                        ))
                    inst.sync_info = mybir.SyncInfo(
                        on_wait=[waits[-1]], on_update=list(si.on_update or []))
                new_insts.append(inst)
            bb.instructions = new_insts
    return n


def make_in_maps(x, viewmats, Ks, w_qkv, w_proj):
    import ml_dtypes
    bft = ml_dtypes.bfloat16
    x = np.asarray(x, np.float32)
    viewmats = np.asarray(viewmats, np.float32)
    Ks = np.asarray(Ks, np.float32)
    w_qkv = np.asarray(w_qkv, np.float32)
    w_proj = np.asarray(w_proj, np.float32)

    csc, css = _rope_tables()
    P, P_inv = _cam_mats(viewmats, Ks)
    w3 = w_qkv.reshape(3, H, DH, DM)
    I32 = np.eye(32, dtype=np.float32)
    ident = np.eye(128, dtype=bft)

    in_maps = []
    for core in range(N_CORES):
        b, hg = divmod(core, HPG)
        heads = slice(4 * hg, 4 * hg + 4)
        xT = np.ascontiguousarray(x[b].T).astype(bft)           # (DM, L)
        wq = np.ascontiguousarray(w3[0, heads].reshape(HD4, DM).T).astype(bft)
        wk = np.ascontiguousarray(w3[1, heads].reshape(HD4, DM).T).astype(bft)
        wv = np.ascontiguousarray(w3[2, heads].reshape(HD4, DM).T).astype(bft)
        wp = np.ascontiguousarray(w_proj[:, 256 * hg:256 * hg + 256].T).astype(bft)
        camq = np.stack([np.kron(I32, P_inv[b, c]) for c in range(CAMS)])
        camk = np.stack([np.kron(I32, P[b, c].T) for c in range(CAMS)])
        camo = np.stack([np.kron(I32, P_inv[b, c].T) for c in range(CAMS)])
        selm = np.zeros((128, 128), np.float32)
        selm[0, 0:64] = 1.0
        selm[64, 64:128] = 1.0
        in_maps.append({
            "sel": selm,
            "xt": xT, "wq": wq, "wk": wk, "wv": wv, "wp": wp,
            "cscf": csc, "cssf": css,
            "camq": camq.astype(np.float32),
            "camk": camk.astype(np.float32),
            "camo": camo.astype(np.float32),
            "ident": ident,
        })
    return in_maps


last_results = None


def kernel(x, viewmats, Ks, w_qkv, w_proj):
    from concourse.bass_utils import run_bass_kernel_spmd
    global last_results
    nc = _build_nc()
    if not getattr(nc, "_waits_split", False):
        _split_multi_waits(nc)
        nc._waits_split = True
    in_maps = make_in_maps(x, viewmats, Ks, w_qkv, w_proj)
    res = run_bass_kernel_spmd(nc, in_maps, core_ids=list(range(N_CORES)))
    last_results = res
    outs = res.results
    y = np.zeros((B, L, DM), np.float32)
    for core in range(N_CORES):
        b = core // HPG
        y[b] += np.asarray(outs[core]["yp"], np.float32)
    return y


# revision 32
# speedup vs baseline: 1.1796x; 1.1796x over previous
# PRoPE attention Trainium2 kernel.
# Sharding: 8 cores = 2 batches x 4 head-groups (4 heads each); host
# pre-transposes x / weights, builds RoPE tables + per-camera 4x4 kron
# matrices, and sums the 4 row-parallel y partials per batch.
#
# Per-core design (398us baseline -> ~305-312us):
#  Phase 1 - qkv projections + PRoPE, engine-balanced: PE does the 8-chunk
#   contraction + per-camera block matmuls (f32r), ScalarE moves PSUM->SBUF,
#   DVE does the 2D-RoPE rotation in f32 (bf16 TT-mult and GpSimd both hit
#   slow paths on this silicon). Wide consolidated DMAs, ordered so the
#   first projection starts ~8us in.
#  Phase 2 - attention, software-pipelined per k-tile so ScalarE's exp
#   stream (the 141us floor) never waits on PE: per tile PE issues av(t)
#   then scores(t+1). PSUM: 2 score slots (128,1024) + 2 softmax
#   accumulators (ones-column denominator trick) = exactly 8 banks.
#   Group boundaries emit the next group's prologue before the previous
#   group's tail. Denominators: reshape (1,1024)->(128,8) via DRAM bounce
#   before DVE reciprocal (8 cyc/elem/lane), broadcast back via DRAM.
#  HAM clock-gate management: the half-array attention matmuls read as
#   ~50% PE activity and decay the clock to 1.2 GHz; dense full-array
#   dummy-matmul bursts (overwritten by the next accumulation's
#   start=True) at group starts/mid-group re-arm 2.4 GHz.
#  Tail - last group's 1/den via exp(-ln d) on the then-idle ScalarE and
#   a PE selector-matmul broadcast (no DMA latency); output projection
#   reuses the score-slot rotation, partials stored bf16.
import functools

import numpy as np

B, L, DM = 2, 2048, 1024
H, DH, NG = 16, 64, 16          # heads, head_dim, groups of 4
CAMS, PER_CAM = 8, 256
PX, PY, IW, IH, ROPE_BASE = 16, 16, 256.0, 256.0, 10000.0
HPG = 4                          # heads per group (per core)
HD4 = HPG * DH                   # 256 cols of qkv per core
N_CORES = 8


def _rope_tables():
    """C and S~ tables in (Dh, PER_CAM) layout, tiled to (128, L)."""
    idx = np.arange(PER_CAM)
    u = ((idx % PX) + 0.5) * (IW / PX)
    v = ((idx // PX) + 0.5) * (IH / PY)
    freqs = (np.float32(ROPE_BASE) ** (-(np.arange(NG, dtype=np.float32)) / np.float32(NG)))
    tu = (u[:, None] * freqs[None, :]).astype(np.float32)   # (P, G)
    tv = (v[:, None] * freqs[None, :]).astype(np.float32)
    ca, sa, cb, sb = np.cos(tu), np.sin(tu), np.cos(tv), np.sin(tv)
    Cq = np.zeros((DH, PER_CAM), np.float32)
    Sq = np.zeros((DH, PER_CAM), np.float32)
    for g in range(NG):
        Cq[4 * g + 0] = ca[:, g]; Cq[4 * g + 1] = ca[:, g]
        Cq[4 * g + 2] = cb[:, g]; Cq[4 * g + 3] = cb[:, g]
        Sq[4 * g + 0] = -sa[:, g]; Sq[4 * g + 1] = sa[:, g]
        Sq[4 * g + 2] = -sb[:, g]; Sq[4 * g + 3] = sb[:, g]
    CqL = np.tile(Cq, (1, CAMS))           # (64, 2048)
    SqL = np.tile(Sq, (1, CAMS))
    return np.tile(CqL, (2, 1)), np.tile(SqL, (2, 1))  # (128, 2048)


def _cam_mats(viewmats, Ks):
    """Per-(batch,cam) P and P^-1."""
    K4 = np.zeros((B, CAMS, 4, 4), np.float32)
    K4[..., :3, :3] = Ks
    K4[..., 3, 3] = 1.0
    P = (K4 @ viewmats).astype(np.float32)
    P_inv = np.linalg.inv(P.astype(np.float64)).astype(np.float32)
    return P, P_inv


@functools.lru_cache(maxsize=1)
def _build_nc():
    import concourse.bass as bass
    import concourse.mybir as mybir
    from concourse.tile import TileContext
    from contextlib import ExitStack

    dt = mybir.dt
    f32 = dt.float32
    f32r = dt.float32r
    bf16 = dt.bfloat16
    ALU = mybir.AluOpType
    ACT = mybir.ActivationFunctionType

    nc = bass.Bass("TRN2", target_bir_lowering=False, debug=False,
                   num_devices=N_CORES)

    xT_d = nc.dram_tensor("xt", [DM, L], bf16, kind="ExternalInput")
    wq_d = nc.dram_tensor("wq", [DM, HD4], bf16, kind="ExternalInput")
    wk_d = nc.dram_tensor("wk", [DM, HD4], bf16, kind="ExternalInput")
    wv_d = nc.dram_tensor("wv", [DM, HD4], bf16, kind="ExternalInput")
    wp_d = nc.dram_tensor("wp", [HD4, DM], bf16, kind="ExternalInput")
    cscf_d = nc.dram_tensor("cscf", [128, L], f32, kind="ExternalInput")
    cssf_d = nc.dram_tensor("cssf", [128, L], f32, kind="ExternalInput")
    camq_d = nc.dram_tensor("camq", [CAMS, 128, 128], f32, kind="ExternalInput")
    camk_d = nc.dram_tensor("camk", [CAMS, 128, 128], f32, kind="ExternalInput")
    camo_d = nc.dram_tensor("camo", [CAMS, 128, 128], f32, kind="ExternalInput")
    ident_d = nc.dram_tensor("ident", [128, 128], bf16, kind="ExternalInput")
    yp_d = nc.dram_tensor("yp", [L, DM], bf16, kind="ExternalOutput")
    bounce_d = nc.dram_tensor("bounce", [8, 1024], f32, kind="Internal")
    bounce2_d = nc.dram_tensor("bounce2", [8, 1024], f32, kind="Internal")

    SWAP_MASK = [i ^ 1 for i in range(32)]

    with TileContext(nc) as tc, ExitStack() as ctx:
        # ---- persistent pools --------------------------------------------
        ptab = ctx.enter_context(tc.tile_pool(name="ptab", bufs=6))
        pcam = ctx.enter_context(tc.tile_pool(name="pcam", bufs=40))
        pqk = ctx.enter_context(tc.tile_pool(name="pqk", bufs=4))
        pvt = ctx.enter_context(tc.tile_pool(name="pvt", bufs=32))
        popt = ctx.enter_context(tc.tile_pool(name="popt", bufs=2))
        pwp = ctx.enter_context(tc.tile_pool(name="pwp", bufs=2))

        # DMA priority: wk -> xt0 -> camk -> tables -> wv -> ...
        cscf = ptab.tile([128, L], f32, tag="cscf", bufs=1)
        cssf = ptab.tile([128, L], f32, tag="cssf", bufs=1)
        ident = ptab.tile([128, 128], bf16, tag="id", bufs=1)

        qp = [pqk.tile([128, L], bf16, tag="qk", bufs=4, name=f"qp{i}") for i in range(2)]
        kp = [pqk.tile([128, L], bf16, tag="qk", bufs=4, name=f"kp{i}") for i in range(2)]
        # v tiles (tok, dh): [A64 | 1 | B64 | 1] = 130 cols
        vt = [[pvt.tile([128, 130], bf16, tag="vt", bufs=32, name=f"vt{i}_{t}") for t in range(16)]
              for i in range(2)]
        opT = [popt.tile([128, L], bf16, tag="opt", bufs=2, name=f"opT{i}") for i in range(2)]

        with tc.tile_pool(name="pxt", bufs=32) as pxt, \
             tc.tile_pool(name="pw", bufs=24) as pw, \
             tc.tile_pool(name="pvp", bufs=2) as pvp, \
             tc.tile_pool(name="ptmp", bufs=16) as ptmp, \
             tc.tile_pool(name="psP", bufs=7, space="PSUM") as psP:

            # ---- phase 1: projections + PRoPE ----------------------------
            def big_w(name, dram):
                tile = pw.tile([128, 8 * HD4], bf16, tag="w", bufs=3, name=name)
                nc.sync.dma_start(
                    tile[:].rearrange("p (d f) -> p d f", d=8),
                    dram[:, :].rearrange("(d p) f -> p d f", d=8))
                return tile

            def big_cam(name, dram):
                tile = pcam.tile([128, 8 * 128], f32r, tag="cam", bufs=4, name=name)
                nc.sync.dma_start(
                    tile[:].rearrange("p (c f) -> p c f", c=8),
                    dram[:, :, :].rearrange("c p f -> p c f").bitcast(f32r))
                return tile

            wk_a = big_w("wk", wk_d)
            xt = [pxt.tile([128, 4096], bf16, tag="xt", bufs=4, name=f"xt{lb}")
                  for lb in range(4)]
            nc.sync.dma_start(
                xt[0][:].rearrange("p (d f) -> p d f", d=8),
                xT_d[:, 0:512].rearrange("(d p) f -> p d f", d=8))
            camk = big_cam("camk", camk_d)
            nc.sync.dma_start(ident[:], ident_d[:, :])
            nc.sync.dma_start(cscf[:], cscf_d[:, :])
            nc.sync.dma_start(cssf[:], cssf_d[:, :])
            wv_a = big_w("wv", wv_d)
            nc.sync.dma_start(
                xt[1][:].rearrange("p (d f) -> p d f", d=8),
                xT_d[:, 512:1024].rearrange("(d p) f -> p d f", d=8))
            camq = big_cam("camq", camq_d)
            wq_a = big_w("wq", wq_d)
            for lb in range(2, 4):
                nc.sync.dma_start(
                    xt[lb][:].rearrange("p (d f) -> p d f", d=8),
                    xT_d[:, 512 * lb:512 * lb + 512].rearrange(
                        "(d p) f -> p d f", d=8))
            camo = big_cam("camo", camo_d)
            wp = [pwp.tile([128, DM], bf16, tag="wp", bufs=2, name=f"wp{i}") for i in range(2)]
            for pt in range(2):
                nc.sync.dma_start(wp[pt][:], wp_d[128 * pt:128 * pt + 128, :])
            # preload the exp table set with a dummy 1-elem activation
            dumm = ptmp.tile([1, 1], bf16, tag="dumm", bufs=1)
            nc.scalar.activation(dumm[:], cscf[0:1, 0:1], ACT.Exp, scale=0.125)

            vp = [pvp.tile([128, L], bf16, tag="vp", bufs=2, name=f"vp{i}") for i in range(2)]

            def prope_block(w8, dest, camm, pt, lb):
                """One (tensor, pt, 512-token) projection + PRoPE block.
                All rotation ops on DVE in f32 (bf16 TT-mult and GpSimd both
                hit slow paths); single camera matmul pair per 256-token cam."""
                lsl = slice(512 * lb, 512 * lb + 512)
                acc = psP.tile([128, 512], f32, tag="acc", bufs=4)
                for d in range(8):
                    wsl = slice(256 * d + 128 * pt, 256 * d + 128 * pt + 128)
                    nc.tensor.matmul(acc[:], w8[:, wsl],
                                     xt[lb][:, 512 * d:512 * d + 512],
                                     start=(d == 0), stop=(d == 7))
                tb = ptmp.tile([128, 512], f32, tag="tb", bufs=3)
                nc.scalar.copy(tb[:], acc[:])
                sw = ptmp.tile([128, 512], f32, tag="sw", bufs=3)
                nc.vector.stream_shuffle(sw[:], tb[:], SWAP_MASK)
                t1 = ptmp.tile([128, 512], f32, tag="t1", bufs=3)
                nc.vector.tensor_tensor(t1[:], cscf[:, lsl], tb[:], op=ALU.mult)
                sw2 = ptmp.tile([128, 512], f32, tag="sw2", bufs=3)
                nc.vector.tensor_tensor(sw2[:], cssf[:, lsl], sw[:], op=ALU.mult)
                dd = ptmp.tile([128, 512], f32r, tag="dd", bufs=3)
                nc.vector.tensor_tensor(dd[:], t1[:], sw2[:], op=ALU.add)
                cc = psP.tile([128, 512], f32, tag="cc", bufs=2)
                for ci in range(2):
                    cam = 2 * lb + ci
                    csl = slice(256 * ci, 256 * ci + 256)
                    nc.tensor.matmul(cc[:, csl],
                                     camm[:, 128 * cam:128 * cam + 128],
                                     dd[:, csl], start=True, stop=True)
                nc.scalar.copy(dest[pt][:, lsl], cc[:])

            # k first (both pt), then v (+ transposes), then q
            for lb in range(4):
                for pt in range(2):
                    prope_block(wk_a, kp, camk, pt, lb)
            for lb in range(4):
                for pt in range(2):
                    prope_block(wv_a, vp, camk, pt, lb)
                    for t in range(4 * lb, 4 * lb + 4):
                        dst = vt[pt][t]
                        nc.gpsimd.memset(dst[:], 1.0)
                        tp = psP.tile([128, 128], bf16, tag="tp", bufs=2)
                        nc.tensor.transpose(tp[:], vp[pt][:, 128 * t:128 * t + 128],
                                            ident[:])
                        nc.vector.tensor_copy(dst[:, 0:64], tp[:, 0:64])
                        nc.vector.tensor_copy(dst[:, 65:129], tp[:, 64:128])
            for bi, lb in enumerate((2, 3, 0, 1)):
                for pt in range(2):
                    if bi in (1, 3) and pt == 0:
                        wacc = psP.tile([128, 512], f32, tag="acc", bufs=4)
                        for _ in range(12):
                            nc.tensor.matmul(wacc[:], ident[:],
                                             xt[lb][:, 0:512],
                                             start=True, stop=True)
                    prope_block(wq_a, qp, camq, pt, lb)
            wacc = psP.tile([128, 512], f32, tag="acc", bufs=4)
            for _ in range(14):
                nc.tensor.matmul(wacc[:], ident[:], xt[0][:, 0:512],
                                 start=True, stop=True)

        with tc.tile_pool(name="pat", bufs=8) as pat, \
             tc.tile_pool(name="psm", bufs=16) as psm, \
             tc.tile_pool(name="pyo", bufs=4) as pyo, \
             tc.tile_pool(name="psS", bufs=2, space="PSUM") as psS, \
             tc.tile_pool(name="psO", bufs=2, space="PSUM") as psO:

            # ---- phase 2: attention, ACT-bound software pipeline ---------
            def S(pt, qg, t, hi, pstiles, warm=False):
                """Scores for k-tile t, head hi: ps = kp_slice.T @ qp (2 MMs).
                warm=True first issues a full-array 128x128 dummy matmul into
                the slot (overwritten by the real scores): the HAM clock gate
                reads the half-array attention matmuls as ~50% activity and
                decays to 1.2 GHz; a full-array MAC burst re-arms 2.4 GHz for
                ~40us."""
                hsl = slice(64 * hi, 64 * hi + 64)
                ksl = slice(128 * t, 128 * t + 128)
                ps = psS.tile([128, 1024], f32, tag="sc", bufs=2,
                              name=f"sc{pt}_{qg}_{t}_{hi}")
                if warm:
                    for _ in range(8):
                        nc.tensor.matmul(ps[:, 0:512], ident[:],
                                         qp[pt][:, 0:512],
                                         start=True, stop=True)
                for qh in range(2):
                    qsl = slice(1024 * qg + 512 * qh, 1024 * qg + 512 * qh + 512)
                    nc.tensor.matmul(ps[:, 512 * qh:512 * qh + 512],
                                     kp[pt][hsl, ksl], qp[pt][hsl, qsl],
                                     start=True, stop=True,
                                     tile_position=(64 * hi, 0))
                pstiles[(t, hi)] = ps

            def E(pt, qg, t, hi, pstiles, attiles):
                at = pat.tile([128, 1024], bf16, tag="at", bufs=8,
                              name=f"at{pt}_{qg}_{t}_{hi}")
                nc.scalar.activation(at[:], pstiles[(t, hi)][:], ACT.Exp,
                                     scale=0.125)
                attiles[(t, hi)] = at

            def A(pt, qg, t, hi, po, attiles):
                for qh in range(2):
                    nc.tensor.matmul(
                        po[hi][0:65, 512 * qh:512 * qh + 512],
                        vt[pt][t][:, 65 * hi:65 * hi + 65],
                        attiles[(t, hi)][:, 512 * qh:512 * qh + 512],
                        start=(t == 0), stop=(t == 15))

            def tail_a(g, pt, qg, po, st, last=False):
                """Denominators + oc copies for a finished group (DVE/DMA).
                Reciprocal runs at 8 cyc/elem per lane, so reshape the (1,1024)
                denominator row to (128,8) via a DRAM bounce before it."""
                oc = psm.tile([128, 1024], f32r, tag="oc", bufs=2, name=f"oc{g}")
                rd = psm.tile([128, 1024], f32, tag="rd", bufs=2, name=f"rd{g}")
                for hi in range(2):
                    dn = psm.tile([1, 1024], f32, tag=f"dn{hi}", bufs=2,
                                  name=f"dn{g}_{hi}")
                    if last:
                        nc.scalar.copy(oc[64 * hi:64 * hi + 64, :],
                                       po[hi][0:64, :])
                    else:
                        nc.vector.tensor_copy(oc[64 * hi:64 * hi + 64, :],
                                              po[hi][0:64, :])
                    nc.vector.tensor_copy(dn[:], po[hi][64:65, :])
                    nc.sync.dma_start(bounce_d[2 * g + hi, :][None, :], dn[:])
                    rc = psm.tile([128, 8], f32, tag=f"rc{hi}", bufs=2,
                                  name=f"rc{g}_{hi}")
                    nc.sync.dma_start(
                        rc[:], bounce_d[2 * g + hi:2 * g + hi + 1, :].rearrange(
                            "a (p f) -> (a p) f", p=128))
                    rc2 = psm.tile([128, 8], f32, tag=f"rc2{hi}", bufs=2,
                                   name=f"rc2{g}_{hi}")
                    nc.vector.reciprocal(rc2[:], rc[:])
                    nc.sync.dma_start(
                        bounce2_d[2 * g + hi, :][None, :].rearrange(
                            "a (p f) -> (a p) f", p=128), rc2[:])
                    nc.sync.dma_start(
                        rd[64 * hi:64 * hi + 64, :],
                        bounce2_d[2 * g + hi, :][None, :].to_broadcast((64, 1024)))
                st["oc"], st["rd"] = oc, rd

            def tail_b(g, pt, qg, po, st, split=False):
                """Mo camera matmuls + D^T rotation + normalize -> opT."""
                oc, rd = st["oc"], st["rd"]
                mo = psS.tile([128, 1024], f32, tag="sc", bufs=2,
                              name=f"mo{g}")
                halves = (range(2) if split else range(1))
                w = 1024 // len(list(halves))
                for qh in halves:
                    hl = slice(w * qh, w * qh + w)
                    gsl = slice(1024 * qg + w * qh, 1024 * qg + w * qh + w)
                    for ci in range(w // 256):
                        cam = 4 * qg + (w * qh) // 256 + ci
                        csl = slice(w * qh + 256 * ci, w * qh + 256 * ci + 256)
                        nc.tensor.matmul(mo[:, csl],
                                         camo[:, 128 * cam:128 * cam + 128],
                                         oc[:, csl].bitcast(f32r),
                                         start=True, stop=True)
                    sw = psm.tile([128, w], f32, tag=f"msw{qh}", bufs=2,
                                  name=f"msw{g}_{qh}")
                    nc.vector.stream_shuffle(sw[:], mo[:, hl], SWAP_MASK)
                    t1 = psm.tile([128, w], f32, tag=f"mt1{qh}", bufs=2,
                                  name=f"mt1{g}_{qh}")
                    nc.vector.tensor_tensor(t1[:], cscf[:, gsl], mo[:, hl],
                                            op=ALU.mult)
                    sw2 = psm.tile([128, w], f32, tag=f"msw2{qh}", bufs=2,
                                   name=f"msw2{g}_{qh}")
                    nc.vector.tensor_tensor(sw2[:], cssf[:, gsl], sw[:],
                                            op=ALU.mult)
                    t2 = psm.tile([128, w], f32, tag=f"mt2{qh}", bufs=2,
                                  name=f"mt2{g}_{qh}")
                    nc.vector.tensor_tensor(t2[:], t1[:], sw2[:],
                                            op=ALU.subtract)
                    nc.vector.tensor_tensor(opT[pt][:, gsl], t2[:], rd[:, hl],
                                            op=ALU.mult)

            def proj_lt(lt):
                tsl = slice(128 * lt, 128 * lt + 128)
                pool_, tag_ = ((psS, "sc") if lt % 2 == 0 else (psO, "po"))
                ys = pool_.tile([128, 1024], f32, tag=tag_, bufs=2, name=f"ys{lt}")
                for nb in range(2):
                    nsl = slice(512 * nb, 512 * nb + 512)
                    nc.tensor.matmul(ys[:, nsl], opT[0][:, tsl], wp[0][:, nsl],
                                     start=True, stop=False)
                    nc.tensor.matmul(ys[:, nsl], opT[1][:, tsl], wp[1][:, nsl],
                                     start=False, stop=True)
                yo = pyo.tile([128, 1024], bf16, tag="yo", bufs=4, name=f"yo{lt}")
                if lt % 2 == 0:
                    nc.scalar.copy(yo[:], ys[:])
                else:
                    nc.vector.tensor_copy(yo[:], ys[:])
                nc.sync.dma_start(yp_d[tsl, :], yo[:])

            groups = [(0, 1), (0, 0), (1, 1), (1, 0)]
            prev = None
            for g, (pt, qg) in enumerate(groups):
                po = [psO.tile([128, 1024], f32, tag="po", bufs=2,
                               name=f"po{g}_{hi}") for hi in range(2)]
                pstiles, attiles = {}, {}
                for hi in range(2):
                    S(pt, qg, 0, hi, pstiles)
                    E(pt, qg, 0, hi, pstiles, attiles)
                if prev is not None:
                    tail_a(*prev)
                for t in range(16):
                    if t == 0:
                        # dense dummy burst: fills the PE queue so it issues
                        # gap-free for a full HAM window and re-arms 2.4 GHz;
                        # av(0) start=True clears the bank afterwards
                        for _ in range(12):
                            nc.tensor.matmul(po[0][:, 0:512], ident[:],
                                             qp[pt][:, 0:512],
                                             start=True, stop=True)
                    for hi in range(2):
                        A(pt, qg, t, hi, po, attiles)
                        if t < 15:
                            S(pt, qg, t + 1, hi, pstiles,
                              warm=(hi == 0 and t + 1 == 8))
                            E(pt, qg, t + 1, hi, pstiles, attiles)
                    if t == 2 and prev is not None:
                        tail_b(*prev)
                prev = (g, pt, qg, po, {})
            # last group: tail + projection, overlapped
            tail_a(*prev, last=True)
            proj_lt(8)
            proj_lt(9)
            tail_b(*prev, split=True)
            for lt in range(10, 16):
                proj_lt(lt)
            for lt in range(0, 8):
                proj_lt(lt)

    return nc


def _split_multi_waits(nc):
    """This walrus build accepts only one sync-wait per instruction; move
    extras onto standalone InstEventSemaphore ops just before."""
    import concourse.mybir as mybir
    n = 0
    for f in nc.m.functions:
        for bb in f.blocks:
            new_insts = []
            for inst in bb.instructions:
                si = inst.sync_info
                if si is not None and si.on_wait and len(si.on_wait) > 1:
                    waits = list(si.on_wait)
                    for w in waits[:-1]:
                        n += 1
                        new_insts.append(mybir.InstEventSemaphore(
                            name=f"I-splitw-{n}", engine=inst.engine,
                            ins=[], outs=[],
                            sync_info=mybir.SyncInfo(on_wait=[w], on_update=[]),
                        ))
                    inst.sync_info = mybir.SyncInfo(
                        on_wait=[waits[-1]], on_update=list(si.on_update or []))
                new_insts.append(inst)
            bb.instructions = new_insts
    return n


def make_in_maps(x, viewmats, Ks, w_qkv, w_proj):
    import ml_dtypes
    bft = ml_dtypes.bfloat16
    x = np.asarray(x, np.float32)
    viewmats = np.asarray(viewmats, np.float32)
    Ks = np.asarray(Ks, np.float32)
    w_qkv = np.asarray(w_qkv, np.float32)
    w_proj = np.asarray(w_proj, np.float32)

    csc, css = _rope_tables()
    P, P_inv = _cam_mats(viewmats, Ks)
    w3 = w_qkv.reshape(3, H, DH, DM)
    I32 = np.eye(32, dtype=np.float32)
    ident = np.eye(128, dtype=bft)

    in_maps = []
    for core in range(N_CORES):
        b, hg = divmod(core, HPG)
        heads = slice(4 * hg, 4 * hg + 4)
        xT = np.ascontiguousarray(x[b].T).astype(bft)           # (DM, L)
        wq = np.ascontiguousarray(w3[0, heads].reshape(HD4, DM).T).astype(bft)
        wk = np.ascontiguousarray(w3[1, heads].reshape(HD4, DM).T).astype(bft)
        wv = np.ascontiguousarray(w3[2, heads].reshape(HD4, DM).T).astype(bft)
        wp = np.ascontiguousarray(w_proj[:, 256 * hg:256 * hg + 256].T).astype(bft)
        camq = np.stack([np.kron(I32, P_inv[b, c]) for c in range(CAMS)])
        camk = np.stack([np.kron(I32, P[b, c].T) for c in range(CAMS)])
        camo = np.stack([np.kron(I32, P_inv[b, c].T) for c in range(CAMS)])
        in_maps.append({
            "xt": xT, "wq": wq, "wk": wk, "wv": wv, "wp": wp,
            "cscf": csc, "cssf": css,
            "camq": camq.astype(np.float32),
            "camk": camk.astype(np.float32),
            "camo": camo.astype(np.float32),
            "ident": ident,
        })
    return in_maps


last_results = None


def kernel(x, viewmats, Ks, w_qkv, w_proj):
    from concourse.bass_utils import run_bass_kernel_spmd
    global last_results
    nc = _build_nc()
    if not getattr(nc, "_waits_split", False):
        _split_multi_waits(nc)
        nc._waits_split = True
    in_maps = make_in_maps(x, viewmats, Ks, w_qkv, w_proj)
    res = run_bass_kernel_spmd(nc, in_maps, core_ids=list(range(N_CORES)))
    last_results = res
    outs = res.results
    y = np.zeros((B, L, DM), np.float32)
    for core in range(N_CORES):
        b = core // HPG
        y[b] += np.asarray(outs[core]["yp"], np.float32)
    return y
